# revision 8
# baseline (speedup 1.0000x reference)
"""Distance-discriminator kernel for 8 Trainium2 cores (bf16 pipeline).

Math (reference): for x [N, D],
    S[d] = sum_j x[j,d];  Q[d] = sum_j x[j,d]^2
    sq[i,d] = Q[d] - 2 x[i,d] S[d] + N x[i,d]^2      (= sum_j (x[j,d]-x[i,d])^2)
    out = log(sqrt(sq) + eps) @ W.T + b

Device formulation: complete the square,
    u = (sqrt(N) x - S/sqrt(N))^2,  sq = u + C,  C = Q - S^2/N = (sum_j u)/N
    logd2 = ln(sq) = Ln(EMC0*u + EMC0*C) + C0
with the C0 centering and eps folded into host-side weights/bias.

Columns d are sharded across the 8 cores (512 each): S, Q stay local, no
mid-kernel communication. Inputs are cast to bf16 on the host, halving HBM
traffic (tolerance 2e-2 leaves ample room; measured ~3e-3).

Engine split (HW-measured rates: DVE ts 4x bf16 / tt 2x, any accumulating
DVE op 1x, ACT 1x dtype-independent, bn_stats 1x):
  S per chunk: DVE pairwise tt-fold 4096->512 at 2x, one 1x reduce.
  chunks 0,2:  ACT Square(scale=sqrt(N), bias=-S/sqrt(N)), accum -> N*C;
               the Ln bias EMC0*C is derived on ACT itself (no DVE stall).
  chunks 1,3:  DVE v = ts at 4x, u = tt at 2x, C = sum(u) via second fold.
  ACT: Ln over every chunk (the scarce resource).
  PE:  bf16 GEMM, PSUM banks packed 2 j-blocks deep (partitions 0:64/64:128).
All DVE/ACT/DMA streams are explicitly dep-chained in hand-scheduled order;
the Tile list scheduler otherwise reorders the S chain behind later chunks.
"""

import numpy as np
import ml_dtypes

import concourse.bacc as bacc
import concourse.bass as bass
import concourse.tile as tile
from concourse import mybir
from concourse.tile import add_dep_helper
from concourse.bass_utils import run_bass_kernel_spmd

N = 4096          # rows
D = 4096          # feature columns
OUT = 64
NCORES = 8
DC = D // NCORES  # 512 columns per core
KCH = DC // 128   # 4 partition-chunks per core
SQRT_N = float(np.sqrt(N))
C0 = 8.9          # ln(sq) centering constant; absorbed via host bias
EMC0 = float(np.exp(-C0))
ZCH = (0, 2)      # chunks squared on ACT (with free C via accum)

F32 = mybir.dt.float32
BF16 = mybir.dt.bfloat16
BF = ml_dtypes.bfloat16
_cache: dict = {}


class Chain:
    """Adds an explicit ordering edge from each op to the previous one."""

    def __init__(self, reason):
        self.prev = None
        self.reason = reason

    def __call__(self, op):
        ins = getattr(op, "ins", None)
        if ins is not None and self.prev is not None:
            add_dep_helper(ins, self.prev, sync=False, reason=self.reason)
        if ins is not None:
            self.prev = ins
        return op


def _build():
    nc = bacc.Bacc(
        "TRN2",
        target_bir_lowering=False,
        debug=False,
        num_devices=NCORES,
    )
    xT = nc.dram_tensor("xT", [DC, N], BF16, kind="ExternalInput").ap()
    wT = nc.dram_tensor("wT", [128, KCH * OUT], BF16, kind="ExternalInput").ap()
    bb = nc.dram_tensor("bb", [128, 1], F32, kind="ExternalInput").ap()
    out = nc.dram_tensor("out", [128, 4 * 512], BF16, kind="ExternalOutput").ap()

    MUL = mybir.AluOpType.mult
    ADD = mybir.AluOpType.add
    LN = mybir.ActivationFunctionType.Ln
    SQUARE = mybir.ActivationFunctionType.Square
    dve = Chain("dve order")
    act = Chain("act order")
    dma_s = Chain("sync dma order")
    dma_a = Chain("scalar dma order")

    with tile.TileContext(nc) as tc:
        with (
            tc.tile_pool(name="wp", bufs=1) as wp,
            tc.tile_pool(name="xp", bufs=KCH) as xp,
            tc.tile_pool(name="up", bufs=KCH) as up,
            tc.tile_pool(name="lp", bufs=KCH) as lp,
            tc.tile_pool(name="st", bufs=KCH) as st,
            tc.tile_pool(name="pp", bufs=4, space="PSUM") as pp,
        ):
            # ---- DMA program: x first on both HWDGE queues, w/bias later
            xs = [xp.tile([128, N], BF16, name=f"x_{k}", tag="x") for k in range(KCH)]

            def dma_piece(k, lo, hi, eng, ch):
                ch(eng.dma_start(
                    xs[k][:, lo:hi], xT[k * 128 : (k + 1) * 128, lo:hi]
                ))

            # Three DMA queues. The scalar engine gets only two early issues
            # (a backed-up HWDGE ring otherwise stalls the engine and delays
            # ACT compute by ~10us); the idle gpsimd SWDGE queue carries the
            # late chunks.
            dma_g = Chain("gpsimd dma order")
            dma_piece(0, 0, 2048, nc.sync, dma_s)
            dma_piece(0, 2048, 4096, nc.scalar, dma_a)
            dma_piece(1, 0, 2048, nc.sync, dma_s)
            dma_piece(1, 2048, 4096, nc.scalar, dma_a)
            dma_piece(2, 0, 2048, nc.gpsimd, dma_g)
            dma_piece(2, 2048, 4096, nc.gpsimd, dma_g)
            dma_piece(3, 0, 2048, nc.sync, dma_s)
            dma_piece(3, 2048, 4096, nc.gpsimd, dma_g)

            w_all = wp.tile([128, KCH * OUT], BF16, name="w_all", tag="w_all")
            dma_s(nc.sync.dma_start(w_all[:], wT))
            bias_b = wp.tile([128, 1], F32, name="bias_b", tag="bias_b")
            dma_s(nc.sync.dma_start(bias_b[:], bb))

            # ---- Ln table preload (the natural-log set also contains Square)
            dumm = wp.tile([128, 1], F32, name="dumm", tag="dumm")
            nc.vector.memset(dumm[:], 1.0)
            dumm2 = wp.tile([128, 1], F32, name="dumm2", tag="dumm2")
            pre_ln = act(nc.scalar.activation(
                dumm2[:], dumm[:], LN, bias=dumm[:], scale=1.0,
            ))

            # ---- per-chunk S via pairwise folds (DVE), hand-ordered
            def s_chain(k):
                f1 = st.tile([128, 2048], BF16, name=f"f1_{k}", tag="f1")
                dve(nc.vector.tensor_tensor(
                    f1[:], xs[k][:, :2048], xs[k][:, 2048:], op=ADD))
                f2 = st.tile([128, 1024], BF16, name=f"f2_{k}", tag="f2")
                dve(nc.vector.tensor_tensor(f2[:], f1[:, :1024], f1[:, 1024:], op=ADD))
                f3 = st.tile([128, 512], BF16, name=f"f3_{k}", tag="f3")
                dve(nc.vector.tensor_tensor(f3[:], f2[:, :512], f2[:, 512:], op=ADD))
                s_k = st.tile([128, 1], F32, name=f"s_{k}", tag="s")
                dve(nc.vector.tensor_reduce(
                    s_k[:], f3[:], axis=mybir.AxisListType.X, op=ADD))
                bA_k = st.tile([128, 1], F32, name=f"bA_{k}", tag="bA")
                dve(nc.vector.tensor_scalar(
                    bA_k[:], s_k[:], -1.0 / SQRT_N, None, op0=MUL))
                return bA_k, s_k

            # C = sum(u)/N via second fold chain (DVE), for the DVE chunks
            def c_chain(k, u_k):
                g1 = st.tile([128, 2048], BF16, name=f"g1_{k}", tag="g1")
                dve(nc.vector.tensor_tensor(g1[:], u_k[:, :2048], u_k[:, 2048:], op=ADD))
                g2 = st.tile([128, 1024], BF16, name=f"g2_{k}", tag="g2")
                dve(nc.vector.tensor_tensor(g2[:], g1[:, :1024], g1[:, 1024:], op=ADD))
                g3 = st.tile([128, 512], BF16, name=f"g3_{k}", tag="g3")
                dve(nc.vector.tensor_tensor(g3[:], g2[:, :512], g2[:, 512:], op=ADD))
                acc_k = st.tile([128, 1], F32, name=f"acc_{k}", tag="acc")
                dve(nc.vector.tensor_reduce(
                    acc_k[:], g3[:], axis=mybir.AxisListType.X, op=ADD))
                biasLn_k = st.tile([128, 1], F32, name=f"bL_{k}", tag="bL")
                dve(nc.vector.tensor_scalar(
                    biasLn_k[:], acc_k[:], EMC0 / N, None, op0=MUL))
                return biasLn_k

            us = [None] * KCH
            biasLns = [None] * KCH
            lnscales = [None] * KCH

            def act_square(k, bA_k):
                u_k = up.tile([128, N], BF16, name=f"u_{k}", tag="u")
                acc_k = st.tile([128, 1], F32, name=f"acc_{k}", tag="acc")
                sq = act(nc.scalar.activation(
                    u_k[:], xs[k][:], SQUARE,
                    bias=bA_k[:], scale=SQRT_N, accum_out=acc_k[:],
                ))
                if k == 0:
                    add_dep_helper(sq.ins, pre_ln.ins, sync=False,
                                   reason="table preload first")
                biasLn_k = st.tile([128, 1], F32, name=f"bL_{k}", tag="bL")
                act(nc.scalar.mul(biasLn_k[:], acc_k[:], EMC0 / N))
                us[k] = u_k
                biasLns[k] = biasLn_k
                lnscales[k] = EMC0

            def dve_square(k, bA_k):
                v_k = st.tile([128, N], BF16, name=f"v_{k}", tag="v")
                dve(nc.vector.tensor_scalar(
                    v_k[:], xs[k][:], SQRT_N, bA_k[:], op0=MUL, op1=ADD))
                u_k = up.tile([128, N], BF16, name=f"u_{k}", tag="u")
                dve(nc.vector.tensor_tensor(u_k[:], v_k[:], v_k[:], op=MUL))
                us[k] = u_k
                lnscales[k] = EMC0

            # DVE order: c0S | c1S | c2S | v1 u1 | g1 | c3S | v3 u3 | g3
            bA0, s0 = s_chain(0)
            bA1, s1 = s_chain(1)
            bA2, s2 = s_chain(2)
            dve_square(1, bA1)
            biasLns[1] = c_chain(1, us[1])
            bA3, s3 = s_chain(3)
            dve_square(3, bA3)
            biasLns[3] = c_chain(3, us[3])

            # ACT order: sq0 bias0 | ln0 | sq2 bias2 | ln1 | ln2 | ln3(pieces)
            psums = [
                pp.tile([128, 512], F32, name=f"ps_{b}", tag="ps")
                for b in range(4)
            ]
            act_square(0, bA0)

            def ln_and_mm(k, bounds):
                l_k = lp.tile([128, N], BF16, name=f"l_{k}", tag="l")
                for p in range(len(bounds) - 1):
                    lo, hi = bounds[p], bounds[p + 1]
                    act(nc.scalar.activation(
                        l_k[:, lo:hi], us[k][:, lo:hi], LN,
                        bias=biasLns[k][:], scale=lnscales[k],
                    ))
                    for j in range(lo // 512, hi // 512):
                        b, h = j // 2, j % 2
                        nc.tensor.matmul(
                            psums[b][h * 64 : (h + 1) * 64, :],
                            lhsT=w_all[:, k * OUT : (k + 1) * OUT],
                            rhs=l_k[:, j * 512 : (j + 1) * 512],
                            start=(k == 0),
                            stop=(k == KCH - 1),
                        )

            ln_and_mm(0, [0, 4096])
            act_square(2, bA2)
            ln_and_mm(1, [0, 4096])
            ln_and_mm(2, [0, 4096])
            ln_and_mm(3, [0, 2048, 3072, 4096])

            out_sb = wp.tile([128, 4 * 512], BF16, name="out_sb", tag="out_sb")
            for b in range(4):
                dve(nc.vector.tensor_scalar(
                    out_sb[:, b * 512 : (b + 1) * 512], psums[b][:],
                    bias_b[:], None, op0=ADD,
                ))
                dma_s(nc.sync.dma_start(
                    out[:, b * 512 : (b + 1) * 512],
                    out_sb[:, b * 512 : (b + 1) * 512],
                ))

    nc.compile()
    return nc


def _prep_inputs(data, W, b):
    data = np.asarray(data, dtype=np.float32)
    W = np.asarray(W, dtype=np.float32)
    b = np.asarray(b, dtype=np.float32)
    dataT = np.ascontiguousarray(data.T)               # [D, N]
    W2T = W.T * 0.5                                    # [D, OUT]
    in_maps = []
    for c in range(NCORES):
        xT_c = dataT[c * DC : (c + 1) * DC].astype(BF)             # [DC, N]
        w_c = W2T[c * DC : (c + 1) * DC, :].astype(BF)             # [DC, OUT]
        wT_c = np.ascontiguousarray(
            w_c.reshape(KCH, 128, OUT).transpose(1, 0, 2).reshape(128, KCH * OUT)
        )
        # bias per core: b/8 plus the centering correction C0*sum_d w2[d,o],
        # stacked twice for the partition-packed PSUM layout
        b8_c = (b / NCORES + C0 * w_c.astype(np.float32).sum(axis=0)).astype(np.float32)
        bb_c = np.ascontiguousarray(
            np.concatenate([b8_c, b8_c]).reshape(128, 1)
        )
        in_maps.append({"xT": xT_c, "wT": wT_c, "bb": bb_c})
    return in_maps


def _run(inputs, trace=False, **kwargs):
    if "nc" not in _cache:
        _cache["nc"] = _build()
    nc = _cache["nc"]
    in_maps = _prep_inputs(inputs["data"], inputs["W"], inputs["b"])
    res = run_bass_kernel_spmd(
        nc, in_maps, core_ids=list(range(NCORES)), trace=trace, **kwargs
    )
    acc = np.zeros((128, 2048), dtype=np.float32)
    for c in range(NCORES):
        acc += np.asarray(res.results[c]["out"]).astype(np.float32)
    outT = np.empty((OUT, N), dtype=np.float32)
    for b in range(4):
        outT[:, (2 * b) * 512 : (2 * b + 1) * 512] = acc[0:64, b * 512 : (b + 1) * 512]
        outT[:, (2 * b + 1) * 512 : (2 * b + 2) * 512] = acc[64:128, b * 512 : (b + 1) * 512]
    return np.ascontiguousarray(outT.T), res


def kernel(data, W, b):
    out, _ = _run({"data": data, "W": W, "b": b})
    return out


# revision 9
# speedup vs baseline: 1.0410x; 1.0410x over previous
"""Distance-discriminator kernel for 8 Trainium2 cores (bf16 pipeline).

Math (reference): for x [N, D],
    S[d] = sum_j x[j,d];  Q[d] = sum_j x[j,d]^2
    sq[i,d] = Q[d] - 2 x[i,d] S[d] + N x[i,d]^2      (= sum_j (x[j,d]-x[i,d])^2)
    out = log(sqrt(sq) + eps) @ W.T + b

Device formulation: complete the square,
    u = (sqrt(N) x - S/sqrt(N))^2,  sq = u + C,  C = Q - S^2/N = (sum_j u)/N
    logd2 = ln(sq) = Ln(EMC0*u + EMC0*C) + C0
with the C0 centering and eps folded into host-side weights/bias.

Columns d are sharded across the 8 cores (512 each): S, Q stay local, no
mid-kernel communication. Inputs are cast to bf16 on the host, halving HBM
traffic (tolerance 2e-2 leaves ample room; measured ~3e-3).

Engine split (HW-measured rates: DVE ts 4x bf16 / tt 2x, any accumulating
DVE op 1x, ACT 1x dtype-independent, bn_stats 1x):
  S per chunk: DVE pairwise tt-fold 4096->512 at 2x, one 1x reduce.
  chunks 0,2:  ACT Square(scale=sqrt(N), bias=-S/sqrt(N)), accum -> N*C;
               the Ln bias EMC0*C is derived on ACT itself (no DVE stall).
  chunks 1,3:  DVE v = ts at 4x, u = tt at 2x, C = sum(u) via second fold.
  ACT: Ln over every chunk (the scarce resource).
  PE:  bf16 GEMM, PSUM banks packed 2 j-blocks deep (partitions 0:64/64:128).
All DVE/ACT/DMA streams are explicitly dep-chained in hand-scheduled order;
the Tile list scheduler otherwise reorders the S chain behind later chunks.
"""

import numpy as np
import ml_dtypes

import concourse.bacc as bacc
import concourse.bass as bass
import concourse.tile as tile
from concourse import mybir
from concourse.tile import add_dep_helper
from concourse.bass_utils import run_bass_kernel_spmd

N = 4096          # rows
D = 4096          # feature columns
OUT = 64
NCORES = 8
DC = D // NCORES  # 512 columns per core
KCH = DC // 128   # 4 partition-chunks per core
SQRT_N = float(np.sqrt(N))
C0 = 8.9          # ln(sq) centering constant; absorbed via host bias
EMC0 = float(np.exp(-C0))
ZCH = (0, 2)      # chunks squared on ACT (with free C via accum)

F32 = mybir.dt.float32
BF16 = mybir.dt.bfloat16
BF = ml_dtypes.bfloat16
_cache: dict = {}


class Chain:
    """Adds an explicit ordering edge from each op to the previous one."""

    def __init__(self, reason):
        self.prev = None
        self.reason = reason

    def __call__(self, op):
        ins = getattr(op, "ins", None)
        if ins is not None and self.prev is not None:
            add_dep_helper(ins, self.prev, sync=False, reason=self.reason)
        if ins is not None:
            self.prev = ins
        return op


def _build():
    nc = bacc.Bacc(
        "TRN2",
        target_bir_lowering=False,
        debug=False,
        num_devices=NCORES,
    )
    xT = nc.dram_tensor("xT", [DC, N], BF16, kind="ExternalInput").ap()
    wT = nc.dram_tensor("wT", [128, KCH * OUT], BF16, kind="ExternalInput").ap()
    bb = nc.dram_tensor("bb", [128, 1], F32, kind="ExternalInput").ap()
    out = nc.dram_tensor("out", [128, 4 * 512], BF16, kind="ExternalOutput").ap()

    MUL = mybir.AluOpType.mult
    ADD = mybir.AluOpType.add
    LN = mybir.ActivationFunctionType.Ln
    SQUARE = mybir.ActivationFunctionType.Square
    dve = Chain("dve order")
    act = Chain("act order")
    dma_s = Chain("sync dma order")
    dma_a = Chain("scalar dma order")

    with tile.TileContext(nc) as tc:
        with (
            tc.tile_pool(name="wp", bufs=1) as wp,
            tc.tile_pool(name="xp", bufs=KCH) as xp,
            tc.tile_pool(name="up", bufs=KCH) as up,
            tc.tile_pool(name="lp", bufs=KCH) as lp,
            tc.tile_pool(name="st", bufs=KCH) as st,
            tc.tile_pool(name="pp", bufs=4, space="PSUM") as pp,
        ):
            # ---- DMA program: x first on both HWDGE queues, w/bias later
            xs = [xp.tile([128, N], BF16, name=f"x_{k}", tag="x") for k in range(KCH)]

            def dma_piece(k, lo, hi, eng, ch):
                ch(eng.dma_start(
                    xs[k][:, lo:hi], xT[k * 128 : (k + 1) * 128, lo:hi]
                ))

            # Three DMA queues. The scalar engine gets only two early issues
            # (a backed-up HWDGE ring otherwise stalls the engine and delays
            # ACT compute by ~10us); the idle gpsimd SWDGE queue carries the
            # late chunks.
            dma_g = Chain("gpsimd dma order")
            dma_piece(0, 0, 2048, nc.sync, dma_s)
            dma_piece(0, 2048, 4096, nc.scalar, dma_a)
            dma_piece(1, 0, 2048, nc.sync, dma_s)
            dma_piece(1, 2048, 4096, nc.scalar, dma_a)
            dma_piece(3, 0, 2048, nc.gpsimd, dma_g)
            dma_piece(3, 2048, 4096, nc.gpsimd, dma_g)
            dma_piece(2, 0, 2048, nc.sync, dma_s)
            dma_piece(2, 2048, 4096, nc.gpsimd, dma_g)

            w_all = wp.tile([128, KCH * OUT], BF16, name="w_all", tag="w_all")
            dma_s(nc.sync.dma_start(w_all[:], wT))
            bias_b = wp.tile([128, 1], F32, name="bias_b", tag="bias_b")
            dma_s(nc.sync.dma_start(bias_b[:], bb))

            # ---- Ln table preload (the natural-log set also contains Square)
            dumm = wp.tile([128, 1], F32, name="dumm", tag="dumm")
            nc.vector.memset(dumm[:], 1.0)
            dumm2 = wp.tile([128, 1], F32, name="dumm2", tag="dumm2")
            pre_ln = act(nc.scalar.activation(
                dumm2[:], dumm[:], LN, bias=dumm[:], scale=1.0,
            ))

            # ---- per-chunk S via pairwise folds (DVE), hand-ordered
            def s_chain(k):
                f1 = st.tile([128, 2048], BF16, name=f"f1_{k}", tag="f1")
                dve(nc.vector.tensor_tensor(
                    f1[:], xs[k][:, :2048], xs[k][:, 2048:], op=ADD))
                f2 = st.tile([128, 1024], BF16, name=f"f2_{k}", tag="f2")
                dve(nc.vector.tensor_tensor(f2[:], f1[:, :1024], f1[:, 1024:], op=ADD))
                f3 = st.tile([128, 512], BF16, name=f"f3_{k}", tag="f3")
                dve(nc.vector.tensor_tensor(f3[:], f2[:, :512], f2[:, 512:], op=ADD))
                s_k = st.tile([128, 1], F32, name=f"s_{k}", tag="s")
                dve(nc.vector.tensor_reduce(
                    s_k[:], f3[:], axis=mybir.AxisListType.X, op=ADD))
                bA_k = st.tile([128, 1], F32, name=f"bA_{k}", tag="bA")
                dve(nc.vector.tensor_scalar(
                    bA_k[:], s_k[:], -1.0 / SQRT_N, None, op0=MUL))
                return bA_k, s_k

            # C = sum(u)/N via second fold chain (DVE), for the DVE chunks
            def c_chain(k, u_k):
                g1 = st.tile([128, 2048], BF16, name=f"g1_{k}", tag="g1")
                dve(nc.vector.tensor_tensor(g1[:], u_k[:, :2048], u_k[:, 2048:], op=ADD))
                g2 = st.tile([128, 1024], BF16, name=f"g2_{k}", tag="g2")
                dve(nc.vector.tensor_tensor(g2[:], g1[:, :1024], g1[:, 1024:], op=ADD))
                g3 = st.tile([128, 512], BF16, name=f"g3_{k}", tag="g3")
                dve(nc.vector.tensor_tensor(g3[:], g2[:, :512], g2[:, 512:], op=ADD))
                acc_k = st.tile([128, 1], F32, name=f"acc_{k}", tag="acc")
                dve(nc.vector.tensor_reduce(
                    acc_k[:], g3[:], axis=mybir.AxisListType.X, op=ADD))
                biasLn_k = st.tile([128, 1], F32, name=f"bL_{k}", tag="bL")
                dve(nc.vector.tensor_scalar(
                    biasLn_k[:], acc_k[:], EMC0 / N, None, op0=MUL))
                return biasLn_k

            us = [None] * KCH
            biasLns = [None] * KCH
            lnscales = [None] * KCH

            def act_square(k, bA_k):
                u_k = up.tile([128, N], BF16, name=f"u_{k}", tag="u")
                acc_k = st.tile([128, 1], F32, name=f"acc_{k}", tag="acc")
                sq = act(nc.scalar.activation(
                    u_k[:], xs[k][:], SQUARE,
                    bias=bA_k[:], scale=SQRT_N, accum_out=acc_k[:],
                ))
                if k == 0:
                    add_dep_helper(sq.ins, pre_ln.ins, sync=False,
                                   reason="table preload first")
                biasLn_k = st.tile([128, 1], F32, name=f"bL_{k}", tag="bL")
                act(nc.scalar.mul(biasLn_k[:], acc_k[:], EMC0 / N))
                us[k] = u_k
                biasLns[k] = biasLn_k
                lnscales[k] = EMC0

            def dve_square(k, bA_k):
                v_k = st.tile([128, N], BF16, name=f"v_{k}", tag="v")
                dve(nc.vector.tensor_scalar(
                    v_k[:], xs[k][:], SQRT_N, bA_k[:], op0=MUL, op1=ADD))
                u_k = up.tile([128, N], BF16, name=f"u_{k}", tag="u")
                dve(nc.vector.tensor_tensor(u_k[:], v_k[:], v_k[:], op=MUL))
                us[k] = u_k
                lnscales[k] = EMC0

            # DVE order: c0S | c1S | c2S | c3S | v3 u3 | g3 (chunk 3 is the
            # only DVE-squared chunk; ACT squares 0..2 and stays gapless)
            bA0, s0 = s_chain(0)
            bA1, s1 = s_chain(1)
            bA2, s2 = s_chain(2)
            bA3, s3 = s_chain(3)
            dve_square(3, bA3)
            biasLns[3] = c_chain(3, us[3])

            # ACT order: sq0 bias0 | ln0 | sq2 bias2 | ln1 | ln2 | ln3(pieces)
            psums = [
                pp.tile([128, 512], F32, name=f"ps_{b}", tag="ps")
                for b in range(4)
            ]
            act_square(0, bA0)
            ln0_done = True

            def ln_and_mm(k, bounds):
                l_k = lp.tile([128, N], BF16, name=f"l_{k}", tag="l")
                for p in range(len(bounds) - 1):
                    lo, hi = bounds[p], bounds[p + 1]
                    act(nc.scalar.activation(
                        l_k[:, lo:hi], us[k][:, lo:hi], LN,
                        bias=biasLns[k][:], scale=lnscales[k],
                    ))
                    for j in range(lo // 512, hi // 512):
                        b, h = j // 2, j % 2
                        nc.tensor.matmul(
                            psums[b][h * 64 : (h + 1) * 64, :],
                            lhsT=w_all[:, k * OUT : (k + 1) * OUT],
                            rhs=l_k[:, j * 512 : (j + 1) * 512],
                            start=(k == 0),
                            stop=(k == KCH - 1),
                        )

            ln_and_mm(0, [0, 4096])
            act_square(1, bA1)
            ln_and_mm(1, [0, 4096])
            act_square(2, bA2)
            ln_and_mm(2, [0, 4096])
            ln_and_mm(3, [0, 2048, 3072, 4096])

            out_sb = wp.tile([128, 4 * 512], BF16, name="out_sb", tag="out_sb")
            for b in range(4):
                dve(nc.vector.tensor_scalar(
                    out_sb[:, b * 512 : (b + 1) * 512], psums[b][:],
                    bias_b[:], None, op0=ADD,
                ))
                dma_s(nc.sync.dma_start(
                    out[:, b * 512 : (b + 1) * 512],
                    out_sb[:, b * 512 : (b + 1) * 512],
                ))

    nc.compile()
    return nc


def _prep_inputs(data, W, b):
    data = np.asarray(data, dtype=np.float32)
    W = np.asarray(W, dtype=np.float32)
    b = np.asarray(b, dtype=np.float32)
    dataT = np.ascontiguousarray(data.T)               # [D, N]
    W2T = W.T * 0.5                                    # [D, OUT]
    in_maps = []
    for c in range(NCORES):
        xT_c = dataT[c * DC : (c + 1) * DC].astype(BF)             # [DC, N]
        w_c = W2T[c * DC : (c + 1) * DC, :].astype(BF)             # [DC, OUT]
        wT_c = np.ascontiguousarray(
            w_c.reshape(KCH, 128, OUT).transpose(1, 0, 2).reshape(128, KCH * OUT)
        )
        # bias per core: b/8 plus the centering correction C0*sum_d w2[d,o],
        # stacked twice for the partition-packed PSUM layout
        b8_c = (b / NCORES + C0 * w_c.astype(np.float32).sum(axis=0)).astype(np.float32)
        bb_c = np.ascontiguousarray(
            np.concatenate([b8_c, b8_c]).reshape(128, 1)
        )
        in_maps.append({"xT": xT_c, "wT": wT_c, "bb": bb_c})
    return in_maps


def _run(inputs, trace=False, **kwargs):
    if "nc" not in _cache:
        _cache["nc"] = _build()
    nc = _cache["nc"]
    in_maps = _prep_inputs(inputs["data"], inputs["W"], inputs["b"])
    res = run_bass_kernel_spmd(
        nc, in_maps, core_ids=list(range(NCORES)), trace=trace, **kwargs
    )
    acc = np.zeros((128, 2048), dtype=np.float32)
    for c in range(NCORES):
        acc += np.asarray(res.results[c]["out"]).astype(np.float32)
    outT = np.empty((OUT, N), dtype=np.float32)
    for b in range(4):
        outT[:, (2 * b) * 512 : (2 * b + 1) * 512] = acc[0:64, b * 512 : (b + 1) * 512]
        outT[:, (2 * b + 1) * 512 : (2 * b + 2) * 512] = acc[64:128, b * 512 : (b + 1) * 512]
    return np.ascontiguousarray(outT.T), res


def kernel(data, W, b):
    out, _ = _run({"data": data, "W": W, "b": b})
    return out
